# revision 39
# baseline (speedup 1.0000x reference)
"""GAT-style aggregation kernel for Trainium2, 8 NeuronCores.

Math (per graph):
  h = X @ W;  s1 = h @ b1;  s2 = h @ b2
  P[i,j] = exp(leaky_relu(s1_i + s2_j, 0.2))
         = u_i*a_j + relu(v_i*b_j - u_i*a_j)          (exact identity)
    where u=e^{s1}, v=e^{0.2 s1}, a=e^{s2}, b=e^{0.2 s2}
  l_i = sum_j P[i,j] = u_i*A + R1_i   (A = sum_j a_j, R1 = rowsum of relu term)
  r_i = 1/l_i
  w_j = sum_i r_i P[i,j] = Q*a_j + sum_i r_i E[i,j]   (Q = sum_i u_i r_i)
  out = elu(sum_j w_j h[j,:]) = elu(v2^T W),  v2 = Q*(X^T a) + X^T w_relu

Sharding: batch dim 16 -> 2 graphs per core, W/b replicated, gather on host.
"""

import numpy as np
from contextlib import ExitStack

B_FULL = 16
N_CORES = 8
B_LOC = B_FULL // N_CORES  # 2
N = 2048
F = 128
NBLK = N // 128  # 16

_CACHE = {}


def _build():
    import concourse.bass as bass
    import concourse.tile as tile
    from concourse import bacc, mybir
    from concourse.masks import make_identity

    f32 = mybir.dt.float32
    AF = mybir.ActivationFunctionType
    ALU = mybir.AluOpType
    f32r = mybir.dt.float32r  # rounded fp32: single-pass PE matmul
    bf16 = mybir.dt.bfloat16

    nc = bacc.Bacc("TRN2", target_bir_lowering=False, debug=False)
    x = nc.dram_tensor("x_local", [B_LOC, N, F], f32, kind="ExternalInput").ap()
    w_in = nc.dram_tensor("w_in", [F, F], f32, kind="ExternalInput").ap()
    b_in = nc.dram_tensor("b_in", [2 * F, 1], f32, kind="ExternalInput").ap()
    out = nc.dram_tensor("out_local", [B_LOC, F], f32, kind="ExternalOutput").ap()
    wscr = nc.dram_tensor("wscratch", [B_LOC, 4, N], f32, kind="Internal").ap()

    with tile.TileContext(nc) as tc, ExitStack() as ctx:
        singles = ctx.enter_context(tc.tile_pool(name="singles", bufs=1))
        sb_xt = ctx.enter_context(tc.tile_pool(name="sb_xt", bufs=4))
        sb_e = ctx.enter_context(tc.tile_pool(name="sb_e", bufs=20))
        sb_r1 = ctx.enter_context(tc.tile_pool(name="sb_r1", bufs=24))
        sb_l = ctx.enter_context(tc.tile_pool(name="sb_l", bufs=8))
        # PSUM is 8 banks total, statically allocated per pool slot:
        #   ps_d: 2 slots x [128,1024] (2 banks each) = 4 banks
        #         (shared via tag "d" by D-halves, transposes, misc psum)
        #   ps_big: 2 slots x [128,1024] (2 banks each) shared by s-pairs/w4
        ps_d = ctx.enter_context(tc.tile_pool(name="ps_d", bufs=2, space="PSUM"))
        ps_big = ctx.enter_context(tc.tile_pool(name="ps_big", bufs=2, space="PSUM"))

        # ---------------- setup ----------------
        identity = singles.tile([128, 128], f32, tag="identity")
        make_identity(nc, identity)
        warm_ps = ps_d.tile([128, 128], f32, tag="d")
        nc.tensor.transpose(warm_ps, identity, identity)
        ones_f = singles.tile([128, 1], f32, tag="ones_f")
        nc.vector.memset(ones_f, 1.0)
        ones_col = singles.tile([128, 1], f32r, tag="ones_col")
        nc.vector.tensor_copy(ones_col, ones_f)
        ones_row = singles.tile([1, 128], f32, tag="ones_row")
        nc.vector.memset(ones_row, 1.0)
        # [0.2; 1.0] per-partition activation scale, [1; -1] sign flipper
        scale2_d = nc.inline_tensor(np.array([[0.2], [1.0]], dtype=np.float32),
                                    name="scale2c")
        scale2 = singles.tile([2, 1], f32, tag="scale2")
        nc.sync.dma_start(scale2, scale2_d.ap())
        sgn2_d = nc.inline_tensor(np.array([[1.0], [-1.0]], dtype=np.float32),
                                  name="sgn2c")
        sgn2 = singles.tile([2, 1], f32, tag="sgn2")
        nc.sync.dma_start(sgn2, sgn2_d.ap())
        warm_s = singles.tile([2, 1], f32, tag="warm_s")
        nc.scalar.copy(warm_s, scale2)
        warm_v = singles.tile([2, 1], f32, tag="warm_v")
        nc.vector.tensor_copy(warm_v, sgn2)

        w_nat = singles.tile([128, 128], f32, tag="w_nat")  # [k, f]
        nc.sync.dma_start(w_nat, w_in)
        wt_ps = ps_d.tile([128, 128], f32, tag="d")
        nc.tensor.transpose(wt_ps, w_nat, identity)  # [f, k]
        wt = singles.tile([128, 128], f32, tag="wt")
        nc.vector.tensor_copy(wt, wt_ps)

        b4 = singles.tile([128, 4], f32, tag="b4")  # cols [b2, b2, b1, b1]
        nc.sync.dma_start(b4[:, 0:1], b_in[128:256, :])
        nc.sync.dma_start(b4[:, 1:2], b_in[128:256, :])
        nc.sync.dma_start(b4[:, 2:3], b_in[0:128, :])
        nc.sync.dma_start(b4[:, 3:4], b_in[0:128, :])
        c4_ps = ps_d.tile([128, 4], f32, tag="d")
        nc.tensor.matmul(c4_ps, lhsT=wt, rhs=b4, start=True, stop=True)
        c4 = singles.tile([128, 4], f32r, tag="c4sb")  # cols [c2, c2, c1, c1]
        nc.vector.tensor_copy(c4, c4_ps)

        xn = singles.tile([128, B_LOC, NBLK, 128], f32, tag="xn")  # [j, bt, blk, k]
        xnr = singles.tile([128, B_LOC, NBLK, 128], f32r, tag="xnr")

        # per-batch persistent tiles
        BA, VU, a_col, u_col, r_col, A_col = {}, {}, {}, {}, {}, {}

        # ---------------- phase 1+2 ----------------
        def phase12(bt):
            # sc cols 0:16 = s2 per block, cols 16:32 = s1 per block
            sc_ps = ps_d.tile([128, 2 * NBLK], f32, tag="d")
            # rows 0:2 hold the data; rows 32:34 hold a DMA replica so a
            # second row-group matmul (tile_position=(32,0)) can run packed
            ba = singles.tile([34, 2048], bf16, tag=f"ba{bt}")  # [b, -a]
            vu = singles.tile([34, 2048], bf16, tag=f"vu{bt}")  # [v, u]
            for half in range(2):
                # rows [s2; s2] and [s1; s1], one column-half at a time
                s2p = ps_big.tile([2, 1024], f32, tag="big")
                s1p = ps_big.tile([2, 1024], f32, tag="big")
                for blk in range(8 * half, 8 * half + 8):
                    col = blk * 128 - half * 1024
                    xslice = xn[:, bt, blk, :]
                    nc.sync.dma_start(xslice, x[bt, blk * 128:(blk + 1) * 128, :])
                    xt_ps = ps_d.tile([128, 128], f32, tag="d")
                    nc.tensor.transpose(xt_ps, xslice, identity)  # [k, n]
                    xt = sb_xt.tile([128, 128], f32r, tag="xt")
                    nc.scalar.copy(xt, xt_ps)
                    nc.gpsimd.tensor_copy(xnr[:, bt, blk, :], xslice)
                    nc.tensor.matmul(s2p[:, col:col + 128],
                                     lhsT=c4[:, 0:2], rhs=xt,
                                     start=True, stop=True)
                    nc.tensor.matmul(s1p[:, col:col + 128],
                                     lhsT=c4[:, 2:4], rhs=xt,
                                     start=True, stop=True)
                    nc.tensor.matmul(sc_ps[:, 2 * blk:2 * blk + 2],
                                     lhsT=xt, rhs=c4[:, 1:3],
                                     start=True, stop=True)
                nc.scalar.activation(ba[0:2, half * 1024:(half + 1) * 1024],
                                     s2p, AF.Exp, scale=scale2)
                nc.scalar.activation(vu[0:2, half * 1024:(half + 1) * 1024],
                                     s1p, AF.Exp, scale=scale2)
            # flip sign of the a row: ba := ba * [1; -1]
            nc.vector.tensor_scalar(ba[0:2, :], ba[0:2, :], sgn2, None, ALU.mult)
            nc.sync.dma_start(ba[32:34, :], ba[0:2, :])
            nc.sync.dma_start(vu[32:34, :], vu[0:2, :])

            # sc cols interleave [s2, s1] per block
            sc3 = sc_ps.rearrange("p (n two) -> p n two", two=2)
            ac = singles.tile([128, NBLK], f32r, tag=f"ac{bt}")
            nc.scalar.activation(ac.rearrange("p (n one) -> p n one", one=1),
                                 sc3[:, :, 0:1], AF.Exp)
            uc = singles.tile([128, NBLK], f32, tag=f"uc{bt}")
            nc.scalar.activation(uc.rearrange("p (n one) -> p n one", one=1),
                                 sc3[:, :, 1:2], AF.Exp)

            # A = sum_j a_j, broadcast to [128,1]
            t16_ps = ps_d.tile([1, NBLK], f32, tag="d")
            nc.tensor.matmul(t16_ps, lhsT=ones_col, rhs=ac, start=True, stop=True)
            t16_sb = singles.tile([1, NBLK], f32, tag=f"t16sb{bt}")
            A1 = singles.tile([1, 1], f32, tag=f"A1{bt}")
            nc.scalar.activation(t16_sb, t16_ps, AF.Identity, accum_out=A1)
            acol_ps = ps_d.tile([128, 1], f32, tag="d")
            nc.tensor.matmul(acol_ps, lhsT=ones_row, rhs=A1, start=True, stop=True)
            Ac = singles.tile([128, 1], f32, tag=f"Acol{bt}")
            nc.vector.tensor_copy(Ac, acol_ps)

            rc = singles.tile([128, NBLK], bf16, tag=f"rc{bt}")
            BA[bt], VU[bt], a_col[bt], u_col[bt], r_col[bt], A_col[bt] = \
                ba, vu, ac, uc, rc, Ac

        # ---------------- phase 3 ----------------
        def phase3(bt):
            ba, vu, uc, rc, Ac = BA[bt], VU[bt], u_col[bt], r_col[bt], A_col[bt]
            # w accumulated col-packed: 4 blocks land on partitions 0/32/64/96
            w4_01 = ps_big.tile([128, 1024], f32, tag="big")  # col quarters 0,1
            w4_23 = ps_big.tile([128, 1024], f32, tag="big")  # col quarters 2,3
            w4 = [w4_01, w4_23]

            def emit_w(group, g4):
                # group: list of 4 (blk, es-halves); g4: group index 0..3
                for q in range(4):
                    for gi, (blk, es) in enumerate(group):
                        nc.tensor.matmul(
                            w4[q // 2][32 * gi:32 * gi + 1,
                                       (q % 2) * 512:(q % 2 + 1) * 512],
                            lhsT=rc[:, blk:blk + 1],
                            rhs=es[q // 2][:, (q % 2) * 512:(q % 2 + 1) * 512],
                            start=(g4 == 0), stop=(g4 == 3),
                            tile_position=(0, 32 * gi),
                            skip_group_check=True)

            def lr_for(blk, r1s):
                lt = sb_l.tile([128, 1], f32, tag="l0")
                nc.vector.scalar_tensor_tensor(
                    out=lt, in0=uc[:, blk:blk + 1], scalar=Ac, in1=r1s[0],
                    op0=ALU.mult, op1=ALU.add)
                for qq in range(1, len(r1s)):
                    ln = sb_l.tile([128, 1], f32, tag=f"l{qq}")
                    nc.vector.tensor_add(ln, lt, r1s[qq])
                    lt = ln
                with nc.allow_low_precision(reason="fp32r r for PE matmul"):
                    nc.vector.reciprocal(rc[:, blk:blk + 1], lt)

            pend = None
            for g4 in range(NBLK // 4):
                group = []
                for pair in range(2):
                    bA, bB = 4 * g4 + 2 * pair, 4 * g4 + 2 * pair + 1
                    vuA = vu[0:2, bA * 128:(bA + 1) * 128]
                    vuB = vu[32:34, bB * 128:(bB + 1) * 128]
                    esA, r1A, esB, r1B = [], [], [], []
                    for h in range(2):
                        cs0 = slice(h * 1024, h * 1024 + 512)
                        cs1 = slice(h * 1024 + 512, h * 1024 + 1024)
                        dA = ps_d.tile([128, 1024], f32, tag="d")
                        nc.tensor.matmul(dA[:, 0:512], lhsT=vuA, rhs=ba[0:2, cs0],
                                         start=True, stop=True)
                        nc.tensor.matmul(dA[:, 512:1024], lhsT=vuA, rhs=ba[0:2, cs1],
                                         start=True, stop=True)
                        dB = ps_d.tile([128, 1024], f32, tag="d")
                        nc.tensor.matmul(dB[:, 0:512], lhsT=vuB, rhs=ba[32:34, cs0],
                                         start=True, stop=True,
                                         tile_position=(32, 0))
                        nc.tensor.matmul(dB[:, 512:1024], lhsT=vuB, rhs=ba[32:34, cs1],
                                         start=True, stop=True,
                                         tile_position=(32, 0))
                        eA = sb_e.tile([128, 1024], bf16, tag="e")
                        r1a = sb_r1.tile([128, 1], f32, tag="r1")
                        nc.scalar.activation(eA, dA, AF.Relu, accum_out=r1a)
                        eB = sb_e.tile([128, 1024], bf16, tag="e")
                        r1b = sb_r1.tile([128, 1], f32, tag="r1")
                        nc.vector.tensor_scalar(eB, dB, 0.0, 0.0, ALU.max,
                                                ALU.add, accum_out=r1b)
                        esA.append(eA); r1A.append(r1a)
                        esB.append(eB); r1B.append(r1b)
                    lr_for(bA, r1A)
                    lr_for(bB, r1B)
                    group.append((bA, esA))
                    group.append((bB, esB))
                if pend is not None:
                    emit_w(*pend)
                pend = (group, g4)
            emit_w(*pend)
            return w4

        # ---------------- tail ----------------
        def tail_a(bt, w4):
            """DMA the col-packed w rows (partitions 0/32/64/96) to DRAM and
            reload in column layout; the 4 quads are summed in tail_b."""
            w4s0 = singles.tile([128, 1024], f32, tag=f"w4s0{bt}")
            nc.scalar.copy(w4s0, w4[0])
            w4s1 = singles.tile([128, 1024], f32, tag=f"w4s1{bt}")
            nc.scalar.copy(w4s1, w4[1])
            nc.sync.dma_start(wscr[bt, :, 0:1024], w4s0[0:128:32, :])
            nc.sync.dma_start(wscr[bt, :, 1024:2048], w4s1[0:128:32, :])
            w_colt4 = singles.tile([128, NBLK, 4], f32, tag=f"wct{bt}")
            for g in range(4):
                nc.sync.dma_start(
                    w_colt4[:, :, g:g + 1],
                    wscr[bt, g:g + 1].rearrange("one (blk p) -> p blk one",
                                                p=128))
            return w_colt4

        def tail_b(bt, w_colt):
            ac, uc, rc = a_col[bt], u_col[bt], r_col[bt]
            ur = singles.tile([128, NBLK], f32r, tag=f"ur{bt}")
            nc.vector.tensor_mul(ur, uc, rc)
            qt_ps = ps_d.tile([1, NBLK], f32, tag="d")
            nc.tensor.matmul(qt_ps, lhsT=ones_col, rhs=ur, start=True, stop=True)
            qt_sb = singles.tile([1, NBLK], f32, tag=f"qtsb{bt}")
            Q1 = singles.tile([1, 1], f32, tag=f"Q1{bt}")
            nc.scalar.activation(qt_sb, qt_ps, AF.Identity, accum_out=Q1)
            qcol_ps = ps_d.tile([128, 1], f32, tag="d")
            nc.tensor.matmul(qcol_ps, lhsT=ones_row, rhs=Q1, start=True, stop=True)
            Qc = singles.tile([128, 1], f32, tag=f"Qcol{bt}")
            nc.vector.tensor_copy(Qc, qcol_ps)

            # cols per block: [a, w0, w1, w2, w3, a] (6 = even, all defined)
            aw = singles.tile([128, NBLK, 6], f32r, tag=f"aw{bt}")
            nc.vector.tensor_copy(aw[:, :, 0:1],
                                  ac.rearrange("p (n one) -> p n one", one=1))
            nc.vector.tensor_copy(aw[:, :, 1:5], w_colt)
            nc.vector.tensor_copy(aw[:, :, 5:6],
                                  ac.rearrange("p (n one) -> p n one", one=1))
            avw_ps = ps_d.tile([128, 6], f32, tag="d")
            for blk in range(NBLK):
                nc.tensor.matmul(avw_ps, lhsT=xnr[:, bt, blk, :],
                                 rhs=aw[:, blk, :],
                                 start=(blk == 0), stop=(blk == NBLK - 1),
                                 skip_group_check=True)
            avw_sb = singles.tile([128, 6], f32, tag=f"avwsb{bt}")
            nc.vector.tensor_copy(avw_sb, avw_ps)
            t1 = singles.tile([128, 1], f32, tag=f"t1{bt}")
            nc.vector.tensor_add(t1, avw_sb[:, 1:2], avw_sb[:, 2:3])
            t2 = singles.tile([128, 1], f32, tag=f"t2{bt}")
            nc.vector.tensor_add(t2, avw_sb[:, 3:4], avw_sb[:, 4:5])
            vw_sb = singles.tile([128, 1], f32, tag=f"vwsb{bt}")
            nc.vector.tensor_add(vw_sb, t1, t2)
            v2 = singles.tile([128, 1], f32, tag=f"v2{bt}")
            nc.vector.scalar_tensor_tensor(
                out=v2, in0=avw_sb[:, 0:1], scalar=Qc, in1=vw_sb,
                op0=ALU.mult, op1=ALU.add)

            res_ps = ps_d.tile([1, 128], f32, tag="d")
            nc.tensor.matmul(res_ps, lhsT=v2, rhs=w_nat, start=True, stop=True)
            # elu: t=relu(x); z=x-t; out = (exp(z)-1)+t
            t = singles.tile([1, 128], f32, tag=f"t{bt}")
            nc.scalar.activation(t, res_ps, AF.Relu)
            z = singles.tile([1, 128], f32, tag=f"z{bt}")
            nc.vector.tensor_scalar(z, res_ps, 0.0, None, ALU.min)
            e1 = singles.tile([1, 128], f32, tag=f"e1{bt}")
            nc.scalar.activation(e1, z, AF.Exp)
            res = singles.tile([1, 128], f32, tag=f"res{bt}")
            nc.vector.scalar_tensor_tensor(
                out=res, in0=e1, scalar=-1.0, in1=t, op0=ALU.add, op1=ALU.add)
            nc.sync.dma_start(out[bt:bt + 1, :], res)

        for bt in range(B_LOC):
            phase12(bt)
        wcols = []
        for bt in range(B_LOC):
            w_ps = phase3(bt)
            wcols.append(tail_a(bt, w_ps))
        for bt in range(B_LOC):
            tail_b(bt, wcols[bt])

    nc.compile()
    return nc


def _ensure_ntff_hook():
    import sys, types
    try:
        import antenv.axon_hooks  # noqa: F401
        return
    except ImportError:
        pass
    mod = types.ModuleType("antenv.axon_hooks")
    _h = {"h": None}
    mod.set_axon_ntff_profile_hook = lambda h: _h.__setitem__("h", h)
    mod.get_axon_ntff_profile_hook = lambda: _h["h"]
    sys.modules["antenv.axon_hooks"] = mod
    from trn_agent_boot.trn_boot import _ntff_profile_via_ctypes
    hook = _ntff_profile_via_ctypes("/opt/axon/libaxon_pjrt.so")
    if hook is not None:
        mod.set_axon_ntff_profile_hook(hook)


def kernel(graphs_feature, W, b):
    graphs_feature = np.ascontiguousarray(graphs_feature, dtype=np.float32)
    W = np.ascontiguousarray(W, dtype=np.float32)
    b = np.ascontiguousarray(b, dtype=np.float32)

    if "nc" not in _CACHE:
        _CACHE["nc"] = _build()
    nc = _CACHE["nc"]

    from concourse.bass_utils import run_bass_kernel_spmd

    in_maps = []
    for c in range(N_CORES):
        in_maps.append({
            "x_local": np.ascontiguousarray(graphs_feature[c * B_LOC:(c + 1) * B_LOC]),
            "w_in": W,
            "b_in": b,
        })
    import os
    trace = bool(os.environ.get("KTRACE"))
    if trace:
        _ensure_ntff_hook()
    r = run_bass_kernel_spmd(nc, in_maps, core_ids=list(range(N_CORES)),
                             trace=trace)
    if trace and r.exec_time_ns is not None:
        print(f"HW exec time: {r.exec_time_ns} ns")
        _CACHE["exec_time_ns"] = r.exec_time_ns
        _CACHE["trace"] = r.instructions_and_trace
        _CACHE["profile_json"] = r.profile_json
    outs = [r.results[c]["out_local"] for c in range(N_CORES)]
    return np.concatenate(outs, axis=0).astype(np.float32)


if __name__ == "__main__":
    nc = _build()
    print("build OK; instructions:", sum(1 for _ in nc.m.functions[0].instructions)
          if hasattr(nc.m.functions[0], "instructions") else "?")
